# revision 24
# baseline (speedup 1.0000x reference)
"""Trainium2 Bass kernel for nn_Classifier_48223892799748 (retrieval_knn).

Computes sim = (D + enc_pm @ cent_pm.T) / 2 where
  enc_pm = sign((samples - 0.5) @ weight.T)  in {+1,-1}
  cent_pm = centroids mapped {0,1} -> {-1,+1}

Sharding: data-parallel over the batch dim (8192 -> 1024 rows per core,
8 cores). weight / centroids replicated.

Device layout: everything is computed transposed ([D, B] / [C, B]) so that
the sign-encoded matmul-1 output tile [128 d, 512 b] feeds matmul-2
directly as the moving operand (contraction over d) with no on-device
transpose.

Matmul-1 precision/speed options:
  USE_F32R=True : single-pass float32r (1s+8e+11m, 1 cyc/row at N=512).
                  Weights (+/-1) are exact in f32r.
  USE_F32R=False: samples hi/lo-split into two bf16 streams accumulated
                  in fp32 PSUM (~fp32 accuracy, 2 cyc/row).
Matmul-2 is exact either way (+/-1 operands in bf16, integer fp32 accum).
"""

import sys

if "/opt/trn_rl_repo" not in sys.path:
    sys.path.insert(0, "/opt/trn_rl_repo")

import ml_dtypes
import numpy as np

import concourse.bass as bass
import concourse.mybir as mybir
import concourse.tile as tile
from concourse import bacc
from concourse.bass_utils import run_bass_kernel_spmd

# The container's `antenv` package is a stub without `axon_hooks`; if tracing
# is ever requested (BASS_TRACE=1), run_bass_kernel_spmd imports it and would
# crash. Provide a stub module (hook=None -> tracing skipped gracefully)
# unless something (e.g. a test harness) registered a real one already.
try:  # pragma: no cover
    import antenv.axon_hooks  # noqa: F401
except ImportError:
    import types as _types

    import antenv as _antenv

    _hooks = _types.ModuleType("antenv.axon_hooks")
    _hook_store = {"h": None}
    _hooks.set_axon_ntff_profile_hook = lambda h: _hook_store.__setitem__("h", h)
    _hooks.get_axon_ntff_profile_hook = lambda: _hook_store["h"]
    sys.modules["antenv.axon_hooks"] = _hooks
    _antenv.axon_hooks = _hooks

BF16 = ml_dtypes.bfloat16

B, IN_F, D, C = 8192, 1024, 10000, 100
N_CORES = 8
B_SH = B // N_CORES          # 1024 batch rows per core
KC = IN_F // 128             # 8 contraction chunks for matmul 1
DT = (D + 127) // 128        # 79 d-tiles
D_PAD = DT * 128             # 10112
NB = B_SH // 512             # 2 psum-width chunks of the local batch
CENTER = 0.5

# matmul-1 mode: "fp8dr" | "f32r" | "bf16_hilo" | "fp16" | "bf16"
# fp8dr: samples quantized to fp8e4m3 (weights +/-1 are exact), DoubleRow
# perf mode contracts 256 k's per instruction at the same 216 ns/matmul as
# a 128-deep bf16/f32r matmul -> mm1 instruction count halves. Measured
# sign-flip rate from sample quantization is 0.84% -> rel err ~9e-3.
import os as _os
MM1_MODE = _os.environ.get("MM1_MODE", "fp8dr")
USE_F32R = MM1_MODE == "f32r"
USE_FP8DR = MM1_MODE == "fp8dr"
KP = IN_F // 256             # 4 k-pair chunks for DoubleRow matmul 1
# matmul-2: fp8e4m3 DoubleRow (2 d-tiles per matmul at 0.5 cyc/row) vs bf16
MM2_DR = _os.environ.get("MM2_DR", "0") == "1"
# matmul-2 as a single fp8-DoubleRow phase AFTER all matmul-1 work (uniform
# PE dtype streams; one mode transition instead of 158)
MM2_PHASE = _os.environ.get("MM2_PHASE", "1") == "1"
NPAIR = (DT + 1) // 2        # 40 d-tile pairs for DoubleRow matmul-2
D_PAD2 = NPAIR * 256         # 10240
C_PAD = 112                  # DoubleRow weight AP needs byte-step %16 == 0

# Stash of the last BassKernelResults (exec_time_ns etc.) for test harnesses.
LAST_RUN = None
_NC_CACHE = None


def _build_nc():
    nc = bacc.Bacc("TRN2", target_bir_lowering=False)
    f32 = mybir.dt.float32
    f32r = mybir.dt.float32r
    bf16 = mybir.dt.bfloat16
    SIGN = mybir.ActivationFunctionType.Sign
    COPY = mybir.ActivationFunctionType.Copy

    # DRAM I/O (per-core shard layouts, see host prep in kernel()):
    #   f32r path:
    #     sf: [128 k_in, KC, B_SH] f32     (samples-0.5).T
    #     wt: [DT, 128 k_in, KC, 128 d_in] f32r  weight.T tiles (+/-1)
    #   bf16 hi/lo path:
    #     sh/sl: [128 k_in, KC, B_SH] bf16 (samples-0.5).T hi/lo
    #     wt:    [DT, 128 k_in, KC, 128 d_in] bf16
    #   ct:  [128 d_in, DT, C] bf16        centroids.T tiles (+/-1)
    #   out: [C, B_SH] f32                 sim.T shard
    fp16 = mybir.dt.float16
    fp8_ = mybir.dt.float8e4
    lp = {
        "f32r": f32r,
        "bf16_hilo": bf16,
        "fp16": fp16,
        "bf16": bf16,
        "fp8dr": fp8_,
    }[MM1_MODE]
    if USE_FP8DR:
        # k-pair layouts for DoubleRow: global k = t*256 + j*128 + k_in
        sf_d = nc.dram_tensor("sf", [128, KP, 2, B_SH], fp8_, kind="ExternalInput")
        wt_d = nc.dram_tensor("wt", [DT, 128, KP, 2, 128], fp8_, kind="ExternalInput")
    elif USE_F32R:
        sf_d = nc.dram_tensor("sf", [128, KC, B_SH], f32, kind="ExternalInput")
        wt_d = nc.dram_tensor("wt", [DT, 128, KC, 128], f32r, kind="ExternalInput")
    elif MM1_MODE == "bf16_hilo":
        sh_d = nc.dram_tensor("sh", [128, KC, B_SH], bf16, kind="ExternalInput")
        sl_d = nc.dram_tensor("sl", [128, KC, B_SH], bf16, kind="ExternalInput")
        wt_d = nc.dram_tensor("wt", [DT, 128, KC, 128], bf16, kind="ExternalInput")
    else:
        sh_d = nc.dram_tensor("sh", [128, KC, B_SH], lp, kind="ExternalInput")
        wt_d = nc.dram_tensor("wt", [DT, 128, KC, 128], lp, kind="ExternalInput")
    fp8 = mybir.dt.float8e4
    if MM2_DR or MM2_PHASE:
        ct_d = nc.dram_tensor("ct", [128, NPAIR, 2, C_PAD], fp8, kind="ExternalInput")
    else:
        ct_d = nc.dram_tensor("ct", [128, DT, C], bf16, kind="ExternalInput")
    if USE_FP8DR:
        # per-class output bias (D - colsum(cent_pm)) / 2: encodings are kept
        # as {0,1} (cheap is_gt on the idle vector engine instead of Sign on
        # the busy scalar engine); agree = 2*(e @ cent_pm) - colsum(cent_pm)
        bv_d = nc.dram_tensor("bv", [C_PAD, 1], f32, kind="ExternalInput")
    out_d = nc.dram_tensor("out", [C, B_SH], f32, kind="ExternalOutput")

    w_dt = lp

    with tile.TileContext(nc) as tc:
        with (
            tc.tile_pool(name="const", bufs=1) as const_pool,
            tc.tile_pool(name="wts", bufs=4) as w_pool,
            tc.tile_pool(name="enc", bufs=3) as enc_pool,
            tc.tile_pool(name="outp", bufs=1) as out_pool,
            tc.tile_pool(name="ps1", bufs=3, space=bass.MemorySpace.PSUM) as ps1_pool,
            tc.tile_pool(name="ps2", bufs=1, space=bass.MemorySpace.PSUM) as ps2_pool,
        ):
            preamble_rest = None
            if USE_FP8DR:
                s_8 = const_pool.tile([128, KP, 2, B_SH], fp8_)
                # head-latency critical path: each DMA_DIRECT2D costs ~610 ns
                # of serial issue time on the sync engine, so order by what
                # the PE needs first and merge the rest into one transfer.
                # w00: tiny duplicate of the first weight pair-tile (32 KB) so
                # the first LDWEIGHTS doesn't wait on the full w[0] load.
                w00 = const_pool.tile([128, 2, 128], fp8_)
                nc.sync.dma_start(w00[:], wt_d[0, :, 0, :, :])
                nc.sync.dma_start(s_8[:, 0, :, : 512], sf_d[:, 0, :, : 512])
                # the other sample chunks issue from the scalar engine's DMA
                # path in parallel with sync's weight-tile issues (each
                # DMA_DIRECT2D costs ~650 ns of serial engine time)
                nc.scalar.dma_start(s_8[:, 0, :, 512:], sf_d[:, 0, :, 512:])
                # kp 1..3 as ONE 768 KB transfer (packets stripe across
                # all 16 DMA engines, so one dma_start loses no bandwidth)
                nc.scalar.dma_start(s_8[:, 1:, :, :], sf_d[:, 1:, :, :])

                def preamble_rest():
                    pass

                s_streams = [s_8]
            elif USE_F32R:
                s_f = const_pool.tile([128, KC, B_SH], f32)
                s_r = const_pool.tile([128, KC, B_SH], f32r)
                # per-kc loads + f32->f32r rounding casts (DVE is otherwise
                # idle); split so PE can start after the first chunk.
                # Only kc=0 is emitted before the d-loop — the rest is
                # deferred until after dt=0's weight DMA so the first weight
                # tile isn't queued behind 6 MB of sample/centroid loads.
                # The kc=0 load/cast is further split by b-chunk so the very
                # first matmul only waits on 256 KB.
                for b in range(NB):
                    nc.sync.dma_start(
                        s_f[:, 0, bass.ts(b, 512)], sf_d[:, 0, bass.ts(b, 512)]
                    )
                    nc.vector.tensor_copy(
                        s_r[:, 0, bass.ts(b, 512)], s_f[:, 0, bass.ts(b, 512)]
                    )

                def preamble_rest():
                    for kc in range(1, KC):
                        nc.sync.dma_start(s_f[:, kc, :], sf_d[:, kc, :])
                        nc.vector.tensor_copy(s_r[:, kc, :], s_f[:, kc, :])

                # tiny duplicate of the first weight sub-tile so the very
                # first matmul waits on 32 KB, not the full 512 KB w[0] load
                w00 = const_pool.tile([128, 128], f32r)
                nc.sync.dma_start(w00[:], wt_d[0, :, 0, :])
                s_streams = [s_r]
            elif MM1_MODE == "bf16_hilo":
                s_hi = const_pool.tile([128, KC, B_SH], bf16)
                s_lo = const_pool.tile([128, KC, B_SH], bf16)
                for kc in range(KC):
                    nc.sync.dma_start(s_hi[:, kc, :], sh_d[:, kc, :])
                    nc.sync.dma_start(s_lo[:, kc, :], sl_d[:, kc, :])
                s_streams = [s_hi, s_lo]
            else:
                s_hi = const_pool.tile([128, KC, B_SH], lp)
                for kc in range(KC):
                    nc.sync.dma_start(s_hi[:, kc, :], sh_d[:, kc, :])
                s_streams = [s_hi]
            if MM2_DR or MM2_PHASE:
                cent = const_pool.tile([128, NPAIR, 2, C_PAD], fp8)
            else:
                cent = const_pool.tile([128, DT, C], bf16)
            if USE_FP8DR:
                bv = const_pool.tile([C_PAD, 1], f32)
            if MM2_PHASE:
                # all sign-encodings buffered on-chip; matmul-2 runs as one
                # uniform fp8-DoubleRow block after the f32r stream ends
                enc_all = const_pool.tile([128, NPAIR, 2, B_SH], fp8)
                # phantom j=1 half of the final pair (dt=79 doesn't exist):
                # zero it so 0-weight x garbage(NaN) can't poison the PSUM
                nc.gpsimd.memset(enc_all[:, NPAIR - 1, 1, :], 0.0)

            c_rows = C_PAD if (MM2_DR or MM2_PHASE) else C
            ps2 = [
                ps2_pool.tile([c_rows, 512], mybir.dt.float32, tag=f"ps2_{b}", name=f"ps2_{b}")
                for b in range(NB)
            ]

            # software pipeline: matmul2 for d-tile dt is issued on PE after
            # the matmul1 block of dt+1, so PE never waits on the Sign
            # activation round-trip.
            pending = []

            def flush_pending():
                if MM2_DR:
                    t0, encs = pending.pop(0)
                    for b in range(NB):
                        nc.tensor.matmul(
                            ps2[b][:],
                            cent[:, t0, :, :],
                            encs[b][:],
                            start=(t0 == 0),
                            stop=(t0 == NPAIR - 1),
                            perf_mode=mybir.MatmulPerfMode.DoubleRow,
                        )
                else:
                    dt0, encs = pending.pop(0)
                    for b in range(NB):
                        nc.tensor.matmul(
                            ps2[b][:],
                            cent[:, dt0, :],
                            encs[b][:],
                            start=(dt0 == 0),
                            stop=(dt0 == DT - 1),
                        )

            n_acc = KP if USE_FP8DR else len(s_streams) * KC
            for dt in range(DT):
                if USE_FP8DR:
                    w = w_pool.tile([128, KP, 2, 128], fp8_, tag="w", name=f"w_{dt}")
                    # single 128 KB transfer per tile: halves the sync-engine
                    # DMA_DIRECT2D issue load (packets stripe over 16 engines)
                    nc.sync.dma_start(w[:], wt_d[dt])
                else:
                    w = w_pool.tile([128, KC, 128], w_dt, tag="w", name=f"w_{dt}")
                    # two half-tile DMAs -> two queues deliver each weight tile
                    # in parallel (one 512 KB DMA needs 137 GB/s from one queue)
                    nc.sync.dma_start(w[:, : KC // 2, :], wt_d[dt, :, : KC // 2, :])
                    nc.sync.dma_start(w[:, KC // 2 :, :], wt_d[dt, :, KC // 2 :, :])
                if dt == 0:
                    # deferred preamble: remaining sample chunks
                    if preamble_rest is not None:
                        preamble_rest()
                if dt == (8 if USE_FP8DR else 0):
                    # centroids (1.15 MB) are not needed until the matmul-2
                    # phase; sync program order (after 8 weight tiles) keeps
                    # the transfer out of the critical head window.
                    nc.sync.dma_start(cent[:], ct_d[:])
                    if USE_FP8DR:
                        nc.sync.dma_start(bv[:], bv_d[:])
                ps1 = [
                    ps1_pool.tile(
                        [128, 512], mybir.dt.float32, tag=f"ps1_{b}", name=f"ps1_{dt}_{b}"
                    )
                    for b in range(NB)
                ]
                if USE_FP8DR:
                    for kp in range(KP):
                        w_src = w00 if (dt == 0 and kp == 0) else w[:, kp, :, :]
                        for b in range(NB):
                            nc.tensor.matmul(
                                ps1[b][:],
                                w_src,
                                s_8[:, kp, :, bass.ts(b, 512)],
                                start=(kp == 0),
                                stop=(kp == KP - 1),
                                perf_mode=mybir.MatmulPerfMode.DoubleRow,
                            )
                else:
                    acc = 0
                    for kc in range(KC):
                        w_src = w00 if (USE_F32R and dt == 0 and kc == 0) else w[:, kc, :]
                        for s_t in s_streams:
                            for b in range(NB):
                                nc.tensor.matmul(
                                    ps1[b][:],
                                    w_src,
                                    s_t[:, kc, bass.ts(b, 512)],
                                    start=(acc == 0),
                                    stop=(acc == n_acc - 1),
                                )
                            acc += 1
                if MM2_PHASE:
                    for b in range(NB):
                        if USE_FP8DR:
                            # e = (proj > 0) as {0,1} fp8 on the otherwise-idle
                            # vector engine (frees ps1 faster than the 689 ns
                            # scalar Sign activation; scalar stays free)
                            nc.vector.tensor_scalar(
                                enc_all[:, dt // 2, dt % 2, bass.ts(b, 512)],
                                ps1[b][:],
                                0.0,
                                None,
                                mybir.AluOpType.is_gt,
                            )
                        else:
                            nc.scalar.activation(
                                enc_all[:, dt // 2, dt % 2, bass.ts(b, 512)],
                                ps1[b][:],
                                SIGN,
                            )
                elif MM2_DR:
                    j = dt % 2
                    if j == 0:
                        cur_pair = [
                            enc_pool.tile(
                                [128, 2, 512], fp8, tag=f"enc_{b}", name=f"e_{dt}_{b}"
                            )
                            for b in range(NB)
                        ]
                    for b in range(NB):
                        nc.scalar.activation(cur_pair[b][:, j, :], ps1[b][:], SIGN)
                    if dt == DT - 1 and j == 0:
                        # odd tile of the final pair does not exist: zero it so
                        # 0-weight x garbage(NaN) cannot poison the PSUM
                        for b in range(NB):
                            nc.gpsimd.memset(cur_pair[b][:, 1, :], 0.0)
                    if j == 1 or dt == DT - 1:
                        pending.append((dt // 2, cur_pair))
                    if len(pending) >= 2:
                        flush_pending()
                else:
                    encs = []
                    for b in range(NB):
                        e = enc_pool.tile(
                            [128, 512], bf16, tag=f"enc_{b}", name=f"e_{dt}_{b}"
                        )
                        nc.scalar.activation(e[:], ps1[b][:], SIGN)
                        encs.append(e)
                    pending.append((dt, encs))
                    if len(pending) >= 2:
                        flush_pending()
            while pending:
                flush_pending()
            if MM2_PHASE and USE_FP8DR:
                # b-outer: ps2[0] finishes after its 40 matmuls, so its
                # output activation + store overlap ps2[1]'s matmul stream
                # instead of serializing after it.
                for b in range(NB):
                    for t in range(NPAIR):
                        nc.tensor.matmul(
                            ps2[b][:],
                            cent[:, t, :, :],
                            enc_all[:, t, :, bass.ts(b, 512)],
                            start=(t == 0),
                            stop=(t == NPAIR - 1),
                            perf_mode=mybir.MatmulPerfMode.DoubleRow,
                        )
                    ob = out_pool.tile(
                        [C, 512], mybir.dt.float32, tag=f"ob_{b}", name=f"ob_{b}"
                    )
                    nc.scalar.activation(
                        ob[:],
                        ps2[b][:C, :],
                        mybir.ActivationFunctionType.Identity,
                        bias=bv[:C],
                        scale=1.0,
                    )
                    nc.sync.dma_start(out_d[:, bass.ts(b, 512)], ob[:])
            else:
                if MM2_PHASE:
                    for t in range(NPAIR):
                        for b in range(NB):
                            nc.tensor.matmul(
                                ps2[b][:],
                                cent[:, t, :, :],
                                enc_all[:, t, :, bass.ts(b, 512)],
                                start=(t == 0),
                                stop=(t == NPAIR - 1),
                                perf_mode=mybir.MatmulPerfMode.DoubleRow,
                            )
                for b in range(NB):
                    ob = out_pool.tile(
                        [C, 512], mybir.dt.float32, tag=f"ob_{b}", name=f"ob_{b}"
                    )
                    nc.scalar.activation(
                        ob[:], ps2[b][:C, :], COPY, bias=D / 2.0, scale=0.5
                    )
                    nc.sync.dma_start(out_d[:, bass.ts(b, 512)], ob[:])

    nc.compile()
    return nc


def _get_nc():
    global _NC_CACHE
    if _NC_CACHE is None:
        _NC_CACHE = _build_nc()
    return _NC_CACHE


def kernel(samples, weight, centroids):
    global LAST_RUN
    samples = np.asarray(samples, dtype=np.float32)
    weight = np.asarray(weight, dtype=np.float32)
    centroids = np.asarray(centroids)

    # ---- host-side marshalling (layout + dtype only) ----
    # centered samples, transposed to [IN_F, B]
    scT = (samples - np.float32(CENTER)).T

    def s_core(a, c):
        # [IN_F, B_SH] -> [128 k_in, KC, B_SH]
        blk = a[:, c * B_SH : (c + 1) * B_SH]
        return np.ascontiguousarray(blk.reshape(KC, 128, B_SH).transpose(1, 0, 2))

    FP16 = np.float16
    FP8 = ml_dtypes.float8_e4m3
    w_np = {
        "f32r": np.float32,
        "bf16_hilo": BF16,
        "fp16": FP16,
        "bf16": BF16,
        "fp8dr": FP8,
    }[MM1_MODE]
    wpad = np.zeros((D_PAD, IN_F), dtype=w_np)
    wpad[:D] = weight.astype(w_np)  # +/-1, exact in bf16/f32r/fp8
    if USE_FP8DR:
        # wt[dt, k_in, t, j, d_in] = weight[dt*128+d_in, t*256+j*128+k_in]
        wt = np.ascontiguousarray(
            wpad.reshape(DT, 128, KP, 2, 128).transpose(0, 4, 2, 3, 1)
        )
    else:
        # weight.T tiles: wt[dt, k_in, kc, d_in] = weight[dt*128+d_in, kc*128+k_in]
        wt = np.ascontiguousarray(wpad.reshape(DT, 128, KC, 128).transpose(0, 3, 2, 1))

    if MM2_DR or MM2_PHASE:
        # DoubleRow centroid tiles: ct[d_in, t, j, c] = cent_pm[c, t*256+j*128+d_in]
        FP8 = ml_dtypes.float8_e4m3
        cpad = np.zeros((D_PAD2, C_PAD), dtype=np.float32)
        cpad[:D, :C] = np.where(centroids, np.float32(1.0), np.float32(-1.0)).T
        ct = np.ascontiguousarray(
            cpad.reshape(NPAIR, 2, 128, C_PAD).transpose(2, 0, 1, 3).astype(FP8)
        )
    else:
        cpad = np.zeros((D_PAD, C), dtype=BF16)
        cpad[:D] = np.where(centroids, np.float32(1.0), np.float32(-1.0)).T.astype(BF16)
        ct = np.ascontiguousarray(cpad.reshape(DT, 128, C).transpose(1, 0, 2))

    if USE_FP8DR:
        s8 = scT.astype(FP8)  # [IN_F, B]

        def s_core8(c):
            # [IN_F, B_SH] -> [128 k_in, KP, 2, B_SH]
            blk = s8[:, c * B_SH : (c + 1) * B_SH]
            return np.ascontiguousarray(
                blk.reshape(KP, 2, 128, B_SH).transpose(2, 0, 1, 3)
            )

        # encodings on device are {0,1}: sim = (D - colsum)/2 + e @ cent_pm.T
        colsum = np.where(centroids, 1.0, -1.0).astype(np.float64).sum(axis=1)
        bvec = np.zeros((C_PAD, 1), dtype=np.float32)
        bvec[:C, 0] = ((D - colsum) / 2.0).astype(np.float32)
        in_maps = [
            {"sf": s_core8(c), "wt": wt, "ct": ct, "bv": bvec}
            for c in range(N_CORES)
        ]
    elif USE_F32R:
        in_maps = [
            {"sf": s_core(scT, c), "wt": wt, "ct": ct} for c in range(N_CORES)
        ]
    elif MM1_MODE == "bf16_hilo":
        s_hi = scT.astype(BF16)
        s_lo = (scT - s_hi.astype(np.float32)).astype(BF16)
        in_maps = [
            {"sh": s_core(s_hi, c), "sl": s_core(s_lo, c), "wt": wt, "ct": ct}
            for c in range(N_CORES)
        ]
    else:
        s_hi = scT.astype(w_np)
        in_maps = [
            {"sh": s_core(s_hi, c), "wt": wt, "ct": ct} for c in range(N_CORES)
        ]

    nc = _get_nc()
    res = run_bass_kernel_spmd(nc, in_maps, core_ids=list(range(N_CORES)))
    LAST_RUN = res

    # gather: out[c] is sim.T for batch rows [c*B_SH, (c+1)*B_SH)
    return np.vstack(
        [np.asarray(res.results[c]["out"]).T for c in range(N_CORES)]
    ).astype(np.float32)



# revision 26
# speedup vs baseline: 1.0031x; 1.0031x over previous
"""Trainium2 Bass kernel for nn_Classifier_48223892799748 (retrieval_knn).

Computes sim = (D + enc_pm @ cent_pm.T) / 2 where
  enc_pm = sign((samples - 0.5) @ weight.T)  in {+1,-1}
  cent_pm = centroids mapped {0,1} -> {-1,+1}

Sharding: data-parallel over the batch dim (8192 -> 1024 rows per core,
8 cores). weight / centroids replicated.

Device layout: everything is computed transposed ([D, B] / [C, B]) so that
the sign-encoded matmul-1 output tile [128 d, 512 b] feeds matmul-2
directly as the moving operand (contraction over d) with no on-device
transpose.

Matmul-1 precision/speed options:
  USE_F32R=True : single-pass float32r (1s+8e+11m, 1 cyc/row at N=512).
                  Weights (+/-1) are exact in f32r.
  USE_F32R=False: samples hi/lo-split into two bf16 streams accumulated
                  in fp32 PSUM (~fp32 accuracy, 2 cyc/row).
Matmul-2 is exact either way (+/-1 operands in bf16, integer fp32 accum).
"""

import sys

if "/opt/trn_rl_repo" not in sys.path:
    sys.path.insert(0, "/opt/trn_rl_repo")

import ml_dtypes
import numpy as np

import concourse.bass as bass
import concourse.mybir as mybir
import concourse.tile as tile
from concourse import bacc
from concourse.bass_utils import run_bass_kernel_spmd

# The container's `antenv` package is a stub without `axon_hooks`; if tracing
# is ever requested (BASS_TRACE=1), run_bass_kernel_spmd imports it and would
# crash. Provide a stub module (hook=None -> tracing skipped gracefully)
# unless something (e.g. a test harness) registered a real one already.
try:  # pragma: no cover
    import antenv.axon_hooks  # noqa: F401
except ImportError:
    import types as _types

    import antenv as _antenv

    _hooks = _types.ModuleType("antenv.axon_hooks")
    _hook_store = {"h": None}
    _hooks.set_axon_ntff_profile_hook = lambda h: _hook_store.__setitem__("h", h)
    _hooks.get_axon_ntff_profile_hook = lambda: _hook_store["h"]
    sys.modules["antenv.axon_hooks"] = _hooks
    _antenv.axon_hooks = _hooks

BF16 = ml_dtypes.bfloat16

B, IN_F, D, C = 8192, 1024, 10000, 100
N_CORES = 8
B_SH = B // N_CORES          # 1024 batch rows per core
KC = IN_F // 128             # 8 contraction chunks for matmul 1
DT = (D + 127) // 128        # 79 d-tiles
D_PAD = DT * 128             # 10112
NB = B_SH // 512             # 2 psum-width chunks of the local batch
CENTER = 0.5

# matmul-1 mode: "fp8dr" | "f32r" | "bf16_hilo" | "fp16" | "bf16"
# fp8dr: samples quantized to fp8e4m3 (weights +/-1 are exact), DoubleRow
# perf mode contracts 256 k's per instruction at the same 216 ns/matmul as
# a 128-deep bf16/f32r matmul -> mm1 instruction count halves. Measured
# sign-flip rate from sample quantization is 0.84% -> rel err ~9e-3.
import os as _os
MM1_MODE = _os.environ.get("MM1_MODE", "fp8dr")
USE_F32R = MM1_MODE == "f32r"
USE_FP8DR = MM1_MODE == "fp8dr"
KP = IN_F // 256             # 4 k-pair chunks for DoubleRow matmul 1
# matmul-2: fp8e4m3 DoubleRow (2 d-tiles per matmul at 0.5 cyc/row) vs bf16
MM2_DR = _os.environ.get("MM2_DR", "0") == "1"
# matmul-2 as a single fp8-DoubleRow phase AFTER all matmul-1 work (uniform
# PE dtype streams; one mode transition instead of 158)
MM2_PHASE = _os.environ.get("MM2_PHASE", "1") == "1"
NPAIR = (DT + 1) // 2        # 40 d-tile pairs for DoubleRow matmul-2
D_PAD2 = NPAIR * 256         # 10240
C_PAD = 112                  # DoubleRow weight AP needs byte-step %16 == 0

# Stash of the last BassKernelResults (exec_time_ns etc.) for test harnesses.
LAST_RUN = None
_NC_CACHE = None


def _build_nc():
    nc = bacc.Bacc("TRN2", target_bir_lowering=False)
    f32 = mybir.dt.float32
    f32r = mybir.dt.float32r
    bf16 = mybir.dt.bfloat16
    SIGN = mybir.ActivationFunctionType.Sign
    COPY = mybir.ActivationFunctionType.Copy

    # DRAM I/O (per-core shard layouts, see host prep in kernel()):
    #   f32r path:
    #     sf: [128 k_in, KC, B_SH] f32     (samples-0.5).T
    #     wt: [DT, 128 k_in, KC, 128 d_in] f32r  weight.T tiles (+/-1)
    #   bf16 hi/lo path:
    #     sh/sl: [128 k_in, KC, B_SH] bf16 (samples-0.5).T hi/lo
    #     wt:    [DT, 128 k_in, KC, 128 d_in] bf16
    #   ct:  [128 d_in, DT, C] bf16        centroids.T tiles (+/-1)
    #   out: [C, B_SH] f32                 sim.T shard
    fp16 = mybir.dt.float16
    fp8_ = mybir.dt.float8e4
    lp = {
        "f32r": f32r,
        "bf16_hilo": bf16,
        "fp16": fp16,
        "bf16": bf16,
        "fp8dr": fp8_,
    }[MM1_MODE]
    if USE_FP8DR:
        # k-pair layouts for DoubleRow: global k = t*256 + j*128 + k_in
        sf_d = nc.dram_tensor("sf", [128, KP, 2, B_SH], fp8_, kind="ExternalInput")
        wt_d = nc.dram_tensor("wt", [DT, 128, KP, 2, 128], fp8_, kind="ExternalInput")
    elif USE_F32R:
        sf_d = nc.dram_tensor("sf", [128, KC, B_SH], f32, kind="ExternalInput")
        wt_d = nc.dram_tensor("wt", [DT, 128, KC, 128], f32r, kind="ExternalInput")
    elif MM1_MODE == "bf16_hilo":
        sh_d = nc.dram_tensor("sh", [128, KC, B_SH], bf16, kind="ExternalInput")
        sl_d = nc.dram_tensor("sl", [128, KC, B_SH], bf16, kind="ExternalInput")
        wt_d = nc.dram_tensor("wt", [DT, 128, KC, 128], bf16, kind="ExternalInput")
    else:
        sh_d = nc.dram_tensor("sh", [128, KC, B_SH], lp, kind="ExternalInput")
        wt_d = nc.dram_tensor("wt", [DT, 128, KC, 128], lp, kind="ExternalInput")
    fp8 = mybir.dt.float8e4
    if MM2_DR or MM2_PHASE:
        ct_d = nc.dram_tensor("ct", [128, NPAIR, 2, C_PAD], fp8, kind="ExternalInput")
    else:
        ct_d = nc.dram_tensor("ct", [128, DT, C], bf16, kind="ExternalInput")
    if USE_FP8DR:
        # per-class output bias (D - colsum(cent_pm)) / 2: encodings are kept
        # as {0,1} (cheap is_gt on the idle vector engine instead of Sign on
        # the busy scalar engine); agree = 2*(e @ cent_pm) - colsum(cent_pm)
        bv_d = nc.dram_tensor("bv", [C_PAD, 1], f32, kind="ExternalInput")
    out_d = nc.dram_tensor("out", [C, B_SH], f32, kind="ExternalOutput")

    w_dt = lp

    with tile.TileContext(nc) as tc:
        with (
            tc.tile_pool(name="const", bufs=1) as const_pool,
            tc.tile_pool(name="wts", bufs=4) as w_pool,
            tc.tile_pool(name="enc", bufs=3) as enc_pool,
            tc.tile_pool(name="outp", bufs=1) as out_pool,
            tc.tile_pool(name="ps1", bufs=3, space=bass.MemorySpace.PSUM) as ps1_pool,
            tc.tile_pool(name="ps2", bufs=1, space=bass.MemorySpace.PSUM) as ps2_pool,
        ):
            preamble_rest = None
            if USE_FP8DR:
                s_8 = const_pool.tile([128, KP, 2, B_SH], fp8_)
                # head-latency critical path: each DMA_DIRECT2D costs ~610 ns
                # of serial issue time on the sync engine, so order by what
                # the PE needs first and merge the rest into one transfer.
                # w00: tiny duplicate of the first weight pair-tile (32 KB) so
                # the first LDWEIGHTS doesn't wait on the full w[0] load.
                w00 = const_pool.tile([128, 2, 128], fp8_)
                nc.sync.dma_start(w00[:], wt_d[0, :, 0, :, :])
                nc.sync.dma_start(s_8[:, 0, :, : 512], sf_d[:, 0, :, : 512])
                # the other sample chunks issue from the scalar engine's DMA
                # path in parallel with sync's weight-tile issues (each
                # DMA_DIRECT2D costs ~650 ns of serial engine time)
                nc.scalar.dma_start(s_8[:, 0, :, 512:], sf_d[:, 0, :, 512:])
                # kp 1..3 as ONE 768 KB transfer (packets stripe across
                # all 16 DMA engines, so one dma_start loses no bandwidth)
                nc.scalar.dma_start(s_8[:, 1:, :, :], sf_d[:, 1:, :, :])

                def preamble_rest():
                    pass

                s_streams = [s_8]
            elif USE_F32R:
                s_f = const_pool.tile([128, KC, B_SH], f32)
                s_r = const_pool.tile([128, KC, B_SH], f32r)
                # per-kc loads + f32->f32r rounding casts (DVE is otherwise
                # idle); split so PE can start after the first chunk.
                # Only kc=0 is emitted before the d-loop — the rest is
                # deferred until after dt=0's weight DMA so the first weight
                # tile isn't queued behind 6 MB of sample/centroid loads.
                # The kc=0 load/cast is further split by b-chunk so the very
                # first matmul only waits on 256 KB.
                for b in range(NB):
                    nc.sync.dma_start(
                        s_f[:, 0, bass.ts(b, 512)], sf_d[:, 0, bass.ts(b, 512)]
                    )
                    nc.vector.tensor_copy(
                        s_r[:, 0, bass.ts(b, 512)], s_f[:, 0, bass.ts(b, 512)]
                    )

                def preamble_rest():
                    for kc in range(1, KC):
                        nc.sync.dma_start(s_f[:, kc, :], sf_d[:, kc, :])
                        nc.vector.tensor_copy(s_r[:, kc, :], s_f[:, kc, :])

                # tiny duplicate of the first weight sub-tile so the very
                # first matmul waits on 32 KB, not the full 512 KB w[0] load
                w00 = const_pool.tile([128, 128], f32r)
                nc.sync.dma_start(w00[:], wt_d[0, :, 0, :])
                s_streams = [s_r]
            elif MM1_MODE == "bf16_hilo":
                s_hi = const_pool.tile([128, KC, B_SH], bf16)
                s_lo = const_pool.tile([128, KC, B_SH], bf16)
                for kc in range(KC):
                    nc.sync.dma_start(s_hi[:, kc, :], sh_d[:, kc, :])
                    nc.sync.dma_start(s_lo[:, kc, :], sl_d[:, kc, :])
                s_streams = [s_hi, s_lo]
            else:
                s_hi = const_pool.tile([128, KC, B_SH], lp)
                for kc in range(KC):
                    nc.sync.dma_start(s_hi[:, kc, :], sh_d[:, kc, :])
                s_streams = [s_hi]
            if MM2_DR or MM2_PHASE:
                cent = const_pool.tile([128, NPAIR, 2, C_PAD], fp8)
            else:
                cent = const_pool.tile([128, DT, C], bf16)
            if USE_FP8DR:
                bv = const_pool.tile([C_PAD, 1], f32)
            if MM2_PHASE:
                # all sign-encodings buffered on-chip; matmul-2 runs as one
                # uniform fp8-DoubleRow block after the f32r stream ends
                enc_all = const_pool.tile([128, NPAIR, 2, B_SH], fp8)
                # phantom j=1 half of the final pair (dt=79 doesn't exist):
                # zero it so 0-weight x garbage(NaN) can't poison the PSUM
                nc.gpsimd.memset(enc_all[:, NPAIR - 1, 1, :], 0.0)

            c_rows = C_PAD if (MM2_DR or MM2_PHASE) else C
            ps2 = [
                ps2_pool.tile([c_rows, 512], mybir.dt.float32, tag=f"ps2_{b}", name=f"ps2_{b}")
                for b in range(NB)
            ]

            # software pipeline: matmul2 for d-tile dt is issued on PE after
            # the matmul1 block of dt+1, so PE never waits on the Sign
            # activation round-trip.
            pending = []

            def flush_pending():
                if MM2_DR:
                    t0, encs = pending.pop(0)
                    for b in range(NB):
                        nc.tensor.matmul(
                            ps2[b][:],
                            cent[:, t0, :, :],
                            encs[b][:],
                            start=(t0 == 0),
                            stop=(t0 == NPAIR - 1),
                            perf_mode=mybir.MatmulPerfMode.DoubleRow,
                        )
                else:
                    dt0, encs = pending.pop(0)
                    for b in range(NB):
                        nc.tensor.matmul(
                            ps2[b][:],
                            cent[:, dt0, :],
                            encs[b][:],
                            start=(dt0 == 0),
                            stop=(dt0 == DT - 1),
                        )

            n_acc = KP if USE_FP8DR else len(s_streams) * KC
            for dt in range(DT):
                if USE_FP8DR:
                    w = w_pool.tile([128, KP, 2, 128], fp8_, tag="w", name=f"w_{dt}")
                    # single 128 KB transfer per tile: halves the sync-engine
                    # DMA_DIRECT2D issue load (packets stripe over 16 engines)
                    nc.sync.dma_start(w[:], wt_d[dt])
                else:
                    w = w_pool.tile([128, KC, 128], w_dt, tag="w", name=f"w_{dt}")
                    # two half-tile DMAs -> two queues deliver each weight tile
                    # in parallel (one 512 KB DMA needs 137 GB/s from one queue)
                    nc.sync.dma_start(w[:, : KC // 2, :], wt_d[dt, :, : KC // 2, :])
                    nc.sync.dma_start(w[:, KC // 2 :, :], wt_d[dt, :, KC // 2 :, :])
                if dt == 0:
                    # deferred preamble: remaining sample chunks
                    if preamble_rest is not None:
                        preamble_rest()
                if dt == (8 if USE_FP8DR else 0):
                    # centroids (1.15 MB) are not needed until the matmul-2
                    # phase; sync program order (after 8 weight tiles) keeps
                    # the transfer out of the critical head window.
                    nc.sync.dma_start(cent[:], ct_d[:])
                    if USE_FP8DR:
                        nc.sync.dma_start(bv[:], bv_d[:])
                ps1 = [
                    ps1_pool.tile(
                        [128, 512], mybir.dt.float32, tag=f"ps1_{b}", name=f"ps1_{dt}_{b}"
                    )
                    for b in range(NB)
                ]
                if USE_FP8DR:
                    for kp in range(KP):
                        w_src = w00 if (dt == 0 and kp == 0) else w[:, kp, :, :]
                        for b in range(NB):
                            nc.tensor.matmul(
                                ps1[b][:],
                                w_src,
                                s_8[:, kp, :, bass.ts(b, 512)],
                                start=(kp == 0),
                                stop=(kp == KP - 1),
                                perf_mode=mybir.MatmulPerfMode.DoubleRow,
                            )
                else:
                    acc = 0
                    for kc in range(KC):
                        w_src = w00 if (USE_F32R and dt == 0 and kc == 0) else w[:, kc, :]
                        for s_t in s_streams:
                            for b in range(NB):
                                nc.tensor.matmul(
                                    ps1[b][:],
                                    w_src,
                                    s_t[:, kc, bass.ts(b, 512)],
                                    start=(acc == 0),
                                    stop=(acc == n_acc - 1),
                                )
                            acc += 1
                if MM2_PHASE:
                    for b in range(NB):
                        if USE_FP8DR and b == 1:
                            # split the encode across engines so both ps1
                            # banks free in parallel: b=1 as {0,1} via is_gt
                            # on the vector engine (bias-corrected at output),
                            # b=0 as +/-1 via Sign on the scalar engine.
                            nc.vector.tensor_scalar(
                                enc_all[:, dt // 2, dt % 2, bass.ts(b, 512)],
                                ps1[b][:],
                                0.0,
                                None,
                                mybir.AluOpType.is_gt,
                            )
                        else:
                            nc.scalar.activation(
                                enc_all[:, dt // 2, dt % 2, bass.ts(b, 512)],
                                ps1[b][:],
                                SIGN,
                            )
                elif MM2_DR:
                    j = dt % 2
                    if j == 0:
                        cur_pair = [
                            enc_pool.tile(
                                [128, 2, 512], fp8, tag=f"enc_{b}", name=f"e_{dt}_{b}"
                            )
                            for b in range(NB)
                        ]
                    for b in range(NB):
                        nc.scalar.activation(cur_pair[b][:, j, :], ps1[b][:], SIGN)
                    if dt == DT - 1 and j == 0:
                        # odd tile of the final pair does not exist: zero it so
                        # 0-weight x garbage(NaN) cannot poison the PSUM
                        for b in range(NB):
                            nc.gpsimd.memset(cur_pair[b][:, 1, :], 0.0)
                    if j == 1 or dt == DT - 1:
                        pending.append((dt // 2, cur_pair))
                    if len(pending) >= 2:
                        flush_pending()
                else:
                    encs = []
                    for b in range(NB):
                        e = enc_pool.tile(
                            [128, 512], bf16, tag=f"enc_{b}", name=f"e_{dt}_{b}"
                        )
                        nc.scalar.activation(e[:], ps1[b][:], SIGN)
                        encs.append(e)
                    pending.append((dt, encs))
                    if len(pending) >= 2:
                        flush_pending()
            while pending:
                flush_pending()
            if MM2_PHASE and USE_FP8DR:
                # b-outer: ps2[0] finishes after its 40 matmuls, so its
                # output activation + store overlap ps2[1]'s matmul stream
                # instead of serializing after it.
                for b in range(NB):
                    for t in range(NPAIR):
                        nc.tensor.matmul(
                            ps2[b][:],
                            cent[:, t, :, :],
                            enc_all[:, t, :, bass.ts(b, 512)],
                            start=(t == 0),
                            stop=(t == NPAIR - 1),
                            perf_mode=mybir.MatmulPerfMode.DoubleRow,
                        )
                    ob = out_pool.tile(
                        [C, 512], mybir.dt.float32, tag=f"ob_{b}", name=f"ob_{b}"
                    )
                    if b == 1:
                        # {0,1} encodings: sim = e @ cent_pm.T + (D-colsum)/2
                        nc.scalar.activation(
                            ob[:],
                            ps2[b][:C, :],
                            mybir.ActivationFunctionType.Identity,
                            bias=bv[:C],
                            scale=1.0,
                        )
                    else:
                        # +/-1 encodings: sim = (D + agree) / 2
                        nc.scalar.activation(
                            ob[:], ps2[b][:C, :], COPY, bias=D / 2.0, scale=0.5
                        )
                    nc.sync.dma_start(out_d[:, bass.ts(b, 512)], ob[:])
            else:
                if MM2_PHASE:
                    for t in range(NPAIR):
                        for b in range(NB):
                            nc.tensor.matmul(
                                ps2[b][:],
                                cent[:, t, :, :],
                                enc_all[:, t, :, bass.ts(b, 512)],
                                start=(t == 0),
                                stop=(t == NPAIR - 1),
                                perf_mode=mybir.MatmulPerfMode.DoubleRow,
                            )
                for b in range(NB):
                    ob = out_pool.tile(
                        [C, 512], mybir.dt.float32, tag=f"ob_{b}", name=f"ob_{b}"
                    )
                    nc.scalar.activation(
                        ob[:], ps2[b][:C, :], COPY, bias=D / 2.0, scale=0.5
                    )
                    nc.sync.dma_start(out_d[:, bass.ts(b, 512)], ob[:])

    nc.compile()
    return nc


def _get_nc():
    global _NC_CACHE
    if _NC_CACHE is None:
        _NC_CACHE = _build_nc()
    return _NC_CACHE


def kernel(samples, weight, centroids):
    global LAST_RUN
    samples = np.asarray(samples, dtype=np.float32)
    weight = np.asarray(weight, dtype=np.float32)
    centroids = np.asarray(centroids)

    # ---- host-side marshalling (layout + dtype only) ----
    # centered samples, transposed to [IN_F, B]
    scT = (samples - np.float32(CENTER)).T

    def s_core(a, c):
        # [IN_F, B_SH] -> [128 k_in, KC, B_SH]
        blk = a[:, c * B_SH : (c + 1) * B_SH]
        return np.ascontiguousarray(blk.reshape(KC, 128, B_SH).transpose(1, 0, 2))

    FP16 = np.float16
    FP8 = ml_dtypes.float8_e4m3
    w_np = {
        "f32r": np.float32,
        "bf16_hilo": BF16,
        "fp16": FP16,
        "bf16": BF16,
        "fp8dr": FP8,
    }[MM1_MODE]
    wpad = np.zeros((D_PAD, IN_F), dtype=w_np)
    wpad[:D] = weight.astype(w_np)  # +/-1, exact in bf16/f32r/fp8
    if USE_FP8DR:
        # wt[dt, k_in, t, j, d_in] = weight[dt*128+d_in, t*256+j*128+k_in]
        wt = np.ascontiguousarray(
            wpad.reshape(DT, 128, KP, 2, 128).transpose(0, 4, 2, 3, 1)
        )
    else:
        # weight.T tiles: wt[dt, k_in, kc, d_in] = weight[dt*128+d_in, kc*128+k_in]
        wt = np.ascontiguousarray(wpad.reshape(DT, 128, KC, 128).transpose(0, 3, 2, 1))

    if MM2_DR or MM2_PHASE:
        # DoubleRow centroid tiles: ct[d_in, t, j, c] = cent_pm[c, t*256+j*128+d_in]
        FP8 = ml_dtypes.float8_e4m3
        cpad = np.zeros((D_PAD2, C_PAD), dtype=np.float32)
        cpad[:D, :C] = np.where(centroids, np.float32(1.0), np.float32(-1.0)).T
        ct = np.ascontiguousarray(
            cpad.reshape(NPAIR, 2, 128, C_PAD).transpose(2, 0, 1, 3).astype(FP8)
        )
    else:
        cpad = np.zeros((D_PAD, C), dtype=BF16)
        cpad[:D] = np.where(centroids, np.float32(1.0), np.float32(-1.0)).T.astype(BF16)
        ct = np.ascontiguousarray(cpad.reshape(DT, 128, C).transpose(1, 0, 2))

    if USE_FP8DR:
        s8 = scT.astype(FP8)  # [IN_F, B]

        def s_core8(c):
            # [IN_F, B_SH] -> [128 k_in, KP, 2, B_SH]
            blk = s8[:, c * B_SH : (c + 1) * B_SH]
            return np.ascontiguousarray(
                blk.reshape(KP, 2, 128, B_SH).transpose(2, 0, 1, 3)
            )

        # encodings on device are {0,1}: sim = (D - colsum)/2 + e @ cent_pm.T
        colsum = np.where(centroids, 1.0, -1.0).astype(np.float64).sum(axis=1)
        bvec = np.zeros((C_PAD, 1), dtype=np.float32)
        bvec[:C, 0] = ((D - colsum) / 2.0).astype(np.float32)
        in_maps = [
            {"sf": s_core8(c), "wt": wt, "ct": ct, "bv": bvec}
            for c in range(N_CORES)
        ]
    elif USE_F32R:
        in_maps = [
            {"sf": s_core(scT, c), "wt": wt, "ct": ct} for c in range(N_CORES)
        ]
    elif MM1_MODE == "bf16_hilo":
        s_hi = scT.astype(BF16)
        s_lo = (scT - s_hi.astype(np.float32)).astype(BF16)
        in_maps = [
            {"sh": s_core(s_hi, c), "sl": s_core(s_lo, c), "wt": wt, "ct": ct}
            for c in range(N_CORES)
        ]
    else:
        s_hi = scT.astype(w_np)
        in_maps = [
            {"sh": s_core(s_hi, c), "wt": wt, "ct": ct} for c in range(N_CORES)
        ]

    nc = _get_nc()
    res = run_bass_kernel_spmd(nc, in_maps, core_ids=list(range(N_CORES)))
    LAST_RUN = res

    # gather: out[c] is sim.T for batch rows [c*B_SH, (c+1)*B_SH)
    return np.vstack(
        [np.asarray(res.results[c]["out"]).T for c in range(N_CORES)]
    ).astype(np.float32)

